# revision 1
# baseline (speedup 1.0000x reference)
"""Trainium2 Bass kernel for nn_ChannelGroupConvUneven.

Computes, for full inputs
    x      (8, 256, 128, 128) f32
    weight (320, 256, 3, 3)   f32
    bias   (320,)             f32
    param  (5,)               i32   per-group input-channel thresholds
the reference
    out = conv2d(x, weight * mask(param), stride 1, VALID) + bias
    out shape (8, 320, 126, 126) f32
where mask zeroes weight[o, i] for i < param[o // 64].

Strategy: data-parallel over batch — one image per NeuronCore (8 cores).
The weight masking + transposition to the matmul lhsT layout is done on the
host (it is tiny); each core runs a dense 3x3 conv as 18 accumulated
matmuls (2 cin blocks x 9 taps) per output tile in float32r (TF32-like)
precision, accumulating in fp32 PSUM.
"""

import numpy as np

import concourse.mybir as mybir
import concourse.tile as tile
from concourse import bacc
from concourse.bass_utils import run_bass_kernel_spmd

N_CORES = 8
P = 128
CIN, COUT, KH, KW = 256, 320, 3, 3
H = W = 128
HO = WO = 126
CB = CIN // P  # 2 cin blocks
RPT = 3  # output rows per matmul tile -> N = 3*126 = 378
BAND_R = 18  # output rows per band
NBANDS = HO // BAND_R  # 7
BAND_IN_R = BAND_R + 2  # input rows needed per band
TPB = BAND_R // RPT  # row-tiles per band (6)
CO_BLOCKS = [(0, 128), (128, 128), (256, 64)]

# float32r: PE "fast fp32" mode (TF32-like rounding, fp32 PSUM accumulation),
# 4x the plain-fp32 matmul rate. Measured rel err ~2e-4 vs fp64 reference.
# Set to mybir.dt.float32 for full fp32 (4 cycles/row instead of 1).
MM_DT = mybir.dt.float32r

_NC_CACHE = {}


def _build_nc(mm_dt):
    nc = bacc.Bacc("TRN2", target_bir_lowering=False, debug=False)
    f32 = mybir.dt.float32

    x_d = nc.dram_tensor("x", [CIN, H, W], mm_dt, kind="ExternalInput").ap()
    w_d = nc.dram_tensor(
        "wt", [P, CB, KH, KW, COUT], mm_dt, kind="ExternalInput"
    ).ap()
    b_d = nc.dram_tensor("biasp", [P, len(CO_BLOCKS)], f32, kind="ExternalInput").ap()
    o_d = nc.dram_tensor("out", [COUT, HO, WO], f32, kind="ExternalOutput").ap()

    # x viewed as [p, cb, h, w]: cin = cb*128 + p
    x_re = x_d.rearrange("(cb p) h w -> p cb h w", p=P)

    with tile.TileContext(nc) as tc:
        with (
            tc.tile_pool(name="wpool", bufs=1) as wpool,
            tc.tile_pool(name="xpool", bufs=2) as xpool,
            tc.tile_pool(name="opool", bufs=6) as opool,
            tc.tile_pool(name="psum", bufs=8, space="PSUM") as psum_pool,
        ):
            wt = wpool.tile([P, CB, KH, KW, COUT], mm_dt)
            nc.sync.dma_start(wt[:], w_d[:])
            bt = wpool.tile([P, len(CO_BLOCKS)], f32)
            nc.sync.dma_start(bt[:], b_d[:])

            for band in range(NBANDS):
                r0 = band * BAND_R  # first output row of band
                xb = xpool.tile([P, CB, BAND_IN_R, W], mm_dt)
                nc.sync.dma_start(xb[:], x_re[:, :, r0 : r0 + BAND_IN_R, :])

                for cob, (co0, com) in enumerate(CO_BLOCKS):
                    for t in range(TPB):
                        rt = t * RPT  # band-relative first output row of tile
                        ps = psum_pool.tile([P, RPT, WO], f32)
                        k = 0
                        for cb in range(CB):
                            for dy in range(KH):
                                for dx in range(KW):
                                    nc.tensor.matmul(
                                        ps[:com],
                                        wt[:, cb, dy, dx, co0 : co0 + com],
                                        xb[:, cb, rt + dy : rt + dy + RPT, dx : dx + WO],
                                        start=(k == 0),
                                        stop=(k == CB * KH * KW - 1),
                                    )
                                    k += 1
                        ot = opool.tile([P, RPT, WO], f32)
                        # evacuate PSUM -> SBUF with fused per-channel bias add
                        nc.scalar.add(ot[:com], ps[:com], bt[:com, cob : cob + 1])
                        nc.sync.dma_start(
                            o_d[co0 : co0 + com, r0 + rt : r0 + rt + RPT, :],
                            ot[:com],
                        )
    nc.compile()
    return nc


def _get_nc():
    key = str(MM_DT)
    if key not in _NC_CACHE:
        _NC_CACHE[key] = _build_nc(MM_DT)
    return _NC_CACHE[key]


def kernel(x, weight, bias, param):
    x = np.ascontiguousarray(np.asarray(x), dtype=np.float32)
    weight = np.asarray(weight, dtype=np.float32)
    bias = np.asarray(bias, dtype=np.float32)
    param = np.asarray(param)

    # host-side weight masking (group g of 64 output channels uses cin >= param[g])
    thresh = np.repeat(param.astype(np.int64), COUT // param.shape[0])  # [COUT]
    mask = (np.arange(CIN)[None, :] >= thresh[:, None]).astype(np.float32)
    wm = weight * mask[:, :, None, None]
    # lhsT layout: [p, cb, kh, kw, cout]
    wT = np.ascontiguousarray(
        wm.reshape(COUT, CB, P, KH, KW).transpose(2, 1, 3, 4, 0)
    )
    biasp = np.zeros((P, len(CO_BLOCKS)), np.float32)
    for j, (co0, com) in enumerate(CO_BLOCKS):
        biasp[:com, j] = bias[co0 : co0 + com]

    nc = _get_nc()
    in_maps = [
        {"x": x[i], "wt": wT, "biasp": biasp} for i in range(N_CORES)
    ]
    res = run_bass_kernel_spmd(nc, in_maps, core_ids=list(range(N_CORES)))
    return np.stack([r["out"] for r in res.results], axis=0)
